# revision 2
# baseline (speedup 1.0000x reference)
"""Trainium2 Bass kernel for sigmoid-projection strictly-causal attention.

Reference computation (B=8, S=2048, D=512, U=512):
    q = sigmoid(x @ Wq); k = sigmoid(x @ Wv); v = sigmoid(x @ Wk)
    score = (q @ k^T) / sqrt(D)                       [S, S]
    mask: strictly causal (key j < query i); row 0 -> zeros
    out = softmax(score) @ v                          [S, U]

Sharding: data-parallel over batch, one batch element per NeuronCore (8
cores). Weights replicated. No collectives.

Per-core kernel (all matmuls in bf16; sigmoid/exp in f32 on ScalarE with
f32 PSUM accumulation):
  1. X^T via PE transposes (d on partitions).
  2. Q^T, K^T = sigmoid(W^T X^T) with u on partitions; V = sigmoid(X W)
     natural [s, u].  Sigmoid fused into the PSUM->SBUF eviction.
  3. Per 128-row query tile i: scores over keys [0, (i+1)*128) only,
     exp (scale 1/sqrt(D) folded in; no max-subtraction needed since
     scores are bounded by sqrt(D)), strict-lower mask on the diagonal
     block, DVE row-sum for the denominator, PE-transposed P blocks feed
     the P @ V accumulation, and the reciprocal denominator is applied on
     the PSUM->SBUF eviction.
"""

import sys

for _p in ("/opt/trn_rl_repo",):
    if _p not in sys.path:
        sys.path.insert(0, _p)

import numpy as np

B, S, D, U = 8, 2048, 512, 512
P = 128
NCORES = 8
DT = D // P  # 4 d-tiles
UT = U // P  # 4 u-tiles
ST = S // P  # 16 s-tiles

_cache = {}


def _build():
    import ml_dtypes
    import concourse.mybir as mybir
    import concourse.tile as tile
    from concourse import bacc

    f32 = mybir.dt.float32
    bf16 = mybir.dt.bfloat16
    AF = mybir.ActivationFunctionType
    ALU = mybir.AluOpType
    AX = mybir.AxisListType

    nc = bacc.Bacc("TRN2", target_bir_lowering=False, debug=False,
                   num_devices=NCORES)

    x_ext = nc.dram_tensor("query", [S, D], f32, kind="ExternalInput")
    wq_ext = nc.dram_tensor("Wq", [D, U], f32, kind="ExternalInput")
    wv_ext = nc.dram_tensor("Wv", [D, U], f32, kind="ExternalInput")
    wk_ext = nc.dram_tensor("Wk", [D, U], f32, kind="ExternalInput")
    out_ext = nc.dram_tensor("out", [S, U], f32, kind="ExternalOutput")

    ident_dram = nc.inline_tensor(
        np.eye(P, dtype=ml_dtypes.bfloat16), "ident_const")
    # [sq_p, sk_f] diagonal block: keep sk < sq  -> strict lower triangle.
    mask_dram = nc.inline_tensor(
        np.tril(np.ones((P, P), np.float32), -1).astype(ml_dtypes.bfloat16),
        "mask_const")

    inv_sqrt_d = 1.0 / float(np.sqrt(D))

    with tile.TileContext(nc) as tc:
        with (
            tc.tile_pool(name="const", bufs=1) as constp,
            tc.tile_pool(name="wpool", bufs=1) as wpool,
            tc.tile_pool(name="stage", bufs=3) as stage,
            tc.tile_pool(name="persist", bufs=1) as persist,
            tc.tile_pool(name="pp", bufs=3) as pp,
            tc.tile_pool(name="ptp", bufs=12) as ptp,
            tc.tile_pool(name="outp", bufs=3) as outp,
            tc.tile_pool(name="small", bufs=8) as smallp,
            tc.tile_pool(name="tpsum", bufs=2, space="PSUM") as tpsum,
            tc.tile_pool(name="mpsum", bufs=2, space="PSUM") as mpsum,
            tc.tile_pool(name="spsum", bufs=2, space="PSUM") as spsum,
            tc.tile_pool(name="opsum", bufs=2, space="PSUM") as opsum,
        ):
            ident = constp.tile([P, P], bf16)
            nc.sync.dma_start(ident[:], ident_dram[:])
            diag_mask = constp.tile([P, P], bf16)
            nc.sync.dma_start(diag_mask[:], mask_dram[:])

            # ---- weights: DMA f32, cast to bf16 ----
            w_bf = {}
            for name, ext in (("q", wq_ext), ("v", wv_ext), ("k", wk_ext)):
                tiles = []
                for t in range(DT):
                    wf = stage.tile([P, U], f32, tag="wstage")
                    nc.sync.dma_start(wf[:], ext[t * P:(t + 1) * P, :])
                    wb = wpool.tile([P, U], bf16, tag=f"w_{name}_{t}")
                    nc.any.tensor_copy(out=wb[:], in_=wf[:])
                    tiles.append(wb)
                w_bf[name] = tiles

            # ---- X^T (bf16, d on partitions): xt[d] is [128, S] ----
            xt = [persist.tile([P, S], bf16, tag=f"xt{d}", name=f"xt{d}") for d in range(DT)]
            for st in range(ST):
                xf = stage.tile([P, D], f32, tag="xstage")
                nc.sync.dma_start(xf[:], x_ext[st * P:(st + 1) * P, :])
                xb = stage.tile([P, D], bf16, tag="xbstage")
                nc.any.tensor_copy(out=xb[:], in_=xf[:])
                for d in range(DT):
                    ps = tpsum.tile([P, P], bf16, tag="tpsum")
                    nc.tensor.transpose(ps[:], xb[:, d * P:(d + 1) * P],
                                        ident[:])
                    nc.any.tensor_copy(out=xt[d][:, st * P:(st + 1) * P],
                                       in_=ps[:])

            # ---- projections ----
            # Q^T, K^T: [u_p, s_f];  out_chunk = sigmoid(W[:,u]^T @ X^T)
            qT = [persist.tile([P, S], bf16, tag=f"qT{u}", name=f"qT{u}") for u in range(UT)]
            kT = [persist.tile([P, S], bf16, tag=f"kT{u}", name=f"kT{u}") for u in range(UT)]
            for u in range(UT):
                for c in range(S // 512):
                    cs = slice(c * 512, (c + 1) * 512)
                    for dst, wkey in ((qT, "q"), (kT, "v")):
                        ps = mpsum.tile([P, 512], f32, tag="mpsum")
                        for d in range(DT):
                            nc.tensor.matmul(
                                ps[:],
                                w_bf[wkey][d][:, u * P:(u + 1) * P],
                                xt[d][:, cs],
                                start=(d == 0), stop=(d == DT - 1))
                        nc.scalar.activation(out=dst[u][:, cs], in_=ps[:],
                                             func=AF.Sigmoid)

            # V natural [s_p, u_f]: v_tile = sigmoid(X^T[:, s]^T @ W)
            vt = [persist.tile([P, U], bf16, tag=f"v{st}", name=f"v{st}") for st in range(ST)]
            for st in range(ST):
                ps = mpsum.tile([P, U], f32, tag="mpsum")
                for d in range(DT):
                    nc.tensor.matmul(
                        ps[:],
                        xt[d][:, st * P:(st + 1) * P],
                        w_bf["k"][d][:],
                        start=(d == 0), stop=(d == DT - 1))
                nc.scalar.activation(out=vt[st][:], in_=ps[:], func=AF.Sigmoid)

            # ---- attention, per query row-tile i ----
            for i in range(ST):
                width = (i + 1) * P  # keys [0, width)
                p_i = pp.tile([P, S], bf16, tag="p")

                # scores + exp, in <=512-wide chunks
                for c0 in range(0, width, 512):
                    w = min(512, width - c0)
                    ps = spsum.tile([P, 512], f32, tag="spsum")
                    for u in range(UT):
                        nc.tensor.matmul(
                            ps[:, :w],
                            qT[u][:, i * P:(i + 1) * P],
                            kT[u][:, c0:c0 + w],
                            start=(u == 0), stop=(u == UT - 1))
                    # p = exp(score / sqrt(D))
                    nc.scalar.activation(out=p_i[:, c0:c0 + w],
                                         in_=ps[:, :w], func=AF.Exp,
                                         scale=inv_sqrt_d)

                # strict-causal mask on the diagonal block
                nc.vector.tensor_mul(out=p_i[:, i * P:(i + 1) * P],
                                     in0=p_i[:, i * P:(i + 1) * P],
                                     in1=diag_mask[:])

                # denominator and its reciprocal
                denom = smallp.tile([P, 1], f32, tag="denom")
                nc.vector.tensor_reduce(denom[:], p_i[:, :width],
                                        axis=AX.X, op=ALU.add)
                # row 0 of tile 0 is fully masked: denom 0 -> keep out at 0
                nc.vector.tensor_scalar_add(denom[:], denom[:], 1e-30)
                recip = smallp.tile([P, 1], f32, tag="recip")
                nc.vector.reciprocal(recip[:], denom[:])

                # P @ V with PE-transposed P blocks
                po = opsum.tile([P, U], f32, tag="opsum")
                for j in range(i + 1):
                    tp = tpsum.tile([P, P], bf16, tag="tpsum")
                    nc.tensor.transpose(tp[:], p_i[:, j * P:(j + 1) * P],
                                        ident[:])
                    pt = ptp.tile([P, P], bf16, tag="pt")
                    nc.any.tensor_copy(out=pt[:], in_=tp[:])
                    nc.tensor.matmul(po[:], pt[:], vt[j][:],
                                     start=(j == 0), stop=(j == i))

                # normalize rows on the way out
                ot = outp.tile([P, U], f32, tag="out")
                nc.vector.tensor_scalar_mul(ot[:], po[:], recip[:, 0:1])
                nc.sync.dma_start(out_ext[i * P:(i + 1) * P, :], ot[:])

    nc.compile()
    return nc


def _get_nc():
    if "nc" not in _cache:
        _cache["nc"] = _build()
    return _cache["nc"]


def kernel(query, Wq, Wv, Wk):
    from concourse.bass_utils import run_bass_kernel_spmd

    nc = _get_nc()
    query = np.ascontiguousarray(query, dtype=np.float32)
    Wq = np.ascontiguousarray(Wq, dtype=np.float32)
    Wv = np.ascontiguousarray(Wv, dtype=np.float32)
    Wk = np.ascontiguousarray(Wk, dtype=np.float32)

    in_maps = [
        {"query": query[b], "Wq": Wq, "Wv": Wv, "Wk": Wk} for b in range(B)
    ]
    res = run_bass_kernel_spmd(nc, in_maps, core_ids=list(range(NCORES)))
    out = np.stack([np.asarray(res.results[b]["out"]) for b in range(B)])
    return out.astype(np.float32)
